# revision 9
# baseline (speedup 1.0000x reference)
"""Trainium2 Bass kernel for ContextualLoss (v2).

Contract: kernel(**inputs) takes FULL inputs {"inputs": [8,128,64,64] f32,
"targets": [8,128,64,64] f32} and returns the FULL scalar loss (np.float32).

Sharding: data-parallel over batch B=8 across the 8 NeuronCores (core b gets
batch element b). Host computes the target channel mean y_mu ([128]) during
input sharding and the final mean of the 8 per-batch losses during gather.

Per-core math (x, y: [C=128, N=4096], mu: [128,1]):
    xc = x - mu ; yc = y - mu                       (bf16)
    v[m] = 1/max(||yc[:,m]||, 1e-12); u[n] likewise for xc
    yv = yc * vrow                                  (v folded pre-matmul)
    Sv = xc^T @ yv                                  (PE, f32 PSUM, 2KB-chunks)
    per 2048-chunk: TTR fuses PSUM->SBUF bf16 copy with row-max accum
      (sv = max(g,g)*1, macc = rowmax)  -> PSUM freed right after matmul
    t = h(1+eps) - h*u*gv ; sc = 1/t ; scale_eff = sc*u ; bias_v = 1/h - sc
    w = exp(scale_eff*sv + bias_v); r = rowsum(w)   (one ACT op, fused accum)
    colmax = max(colmax, w*rinv)  (fused stt: DVE cols [0,S), Pool [S,N))
    loss_b = -log(mean_m colmax_partition_max + eps)

Work split per block: DVE = 2xTTR + reciprocals + stt slice; Pool = scalar
chain + stt main slice; ACT = exp; PE = matmuls. Main loop is software-
pipelined by one block so cross-engine waits don't bubble the DVE queue.
"""

import numpy as np

import concourse.bass as bass
import concourse.tile as tile
from concourse import bacc, masks, mybir
from concourse.bass_utils import run_bass_kernel_spmd
from concourse.dve_ops import TENSOR_MASK_REDUCE

F32 = mybir.dt.float32
BF16 = mybir.dt.bfloat16
AF = mybir.ActivationFunctionType
OP = mybir.AluOpType

B, C, H, W = 8, 128, 64, 64
N = H * W                  # 4096
P = 128                    # partitions / channels
NBLK = N // P              # 32 row blocks
MM_N = 512                 # matmul moving free dim (one PSUM bank)
MC = 2048                  # m-chunk per PSUM tile (4 banks), 2 chunks/block
S_DVE = 640                # colmax stt cols handled by DVE; rest on Pool
H_BW = 0.5
EPS = 1e-5
NORM_EPS = 1e-12
NEG_INF = -3.0e38
N_CORES = 8


def _norm_chain(nc, pool, ssq_ps, name):
    """[128, NBLK] sum-of-squares in PSUM -> inv-norm (f32, SBUF)."""
    nrm = pool.tile([P, NBLK], F32, name=f"nrm_{name}")
    nc.scalar.activation(nrm[:], ssq_ps[:], AF.Sqrt)
    ncl = pool.tile([P, NBLK], F32, name=f"ncl_{name}")
    nc.vector.tensor_scalar_max(ncl[:], nrm[:], NORM_EPS)
    inv = pool.tile([P, NBLK], F32, name=f"inv_{name}")
    nc.vector.reciprocal(inv[:], ncl[:])
    return inv


def _kernel_body(tc):
    nc = tc.nc
    x_d = nc.dram_tensor("x", [P, N], F32, kind="ExternalInput").ap()
    y_d = nc.dram_tensor("y", [P, N], F32, kind="ExternalInput").ap()
    mu_d = nc.dram_tensor("mu", [P, 1], F32, kind="ExternalInput").ap()
    id_d = nc.dram_tensor("ident", [P, P], F32, kind="ExternalInput").ap()
    loss_d = nc.dram_tensor("loss", [1, 1], F32, kind="ExternalOutput").ap()

    from contextlib import ExitStack
    with ExitStack() as ctx:
        persist = ctx.enter_context(tc.tile_pool(name="persist", bufs=1))
        small = ctx.enter_context(tc.tile_pool(name="small", bufs=4))

        # constants
        ident_f = persist.tile([P, P], F32)
        nc.sync.dma_start(ident_f[:], id_d)
        ident_bf = persist.tile([P, P], BF16)
        nc.vector.tensor_copy(ident_bf[:], ident_f[:])
        ones_sq = persist.tile([P, P], BF16)
        nc.vector.memset(ones_sq[:], 1.0)
        ones_col_bf = persist.tile([P, 1], BF16)
        nc.vector.memset(ones_col_bf[:], 1.0)
        ones_col_f = persist.tile([P, 1], F32)
        nc.vector.memset(ones_col_f[:], 1.0)
        c_hbias = persist.tile([P, 1], F32)
        nc.vector.memset(c_hbias[:], H_BW * (1.0 + EPS))
        c_invh = persist.tile([P, 1], F32)
        nc.vector.memset(c_invh[:], 1.0 / H_BW)
        c_eps = persist.tile([P, 1], F32)
        nc.vector.memset(c_eps[:], EPS)
        c_mend = persist.tile([P, 1], F32)   # TMR mask end (covers full chunk)
        nc.vector.memset(c_mend[:], float(N))

        # ---------- load + center ----------
        xc = persist.tile([P, N], BF16)   # centered x, bf16
        yc = persist.tile([P, N], BF16)   # centered y, bf16
        u_col = None
        v_col = None
        with tc.tile_pool(name="load", bufs=1) as load:
            mu_sb = persist.tile([P, 1], F32)
            nc.sync.dma_start(mu_sb[:], mu_d)
            negmu = persist.tile([P, 1], F32)
            nc.vector.tensor_scalar_mul(negmu[:], mu_sb[:], -1.0)
            x_sb = load.tile([P, N], F32)
            y_sb = load.tile([P, N], F32)
            nc.sync.dma_start(x_sb[:], x_d)
            nc.sync.dma_start(y_sb[:], y_d)
            # center: x on DVE, y on ACT (parallel lanes)
            nc.vector.tensor_scalar_sub(xc[:], x_sb[:], mu_sb[:, 0:1])
            nc.scalar.activation(yc[:], y_sb[:], AF.Identity,
                                 bias=negmu[:, 0:1], scale=1.0)

            # squares: x on DVE (bf16 2x), y on ACT
            xsq = load.tile([P, N], BF16)
            nc.vector.tensor_mul(xsq[:], xc[:], xc[:])
            ysq = load.tile([P, N], BF16)
            nc.scalar.activation(ysq[:], yc[:], AF.Square)

            with tc.tile_pool(name="ssq_ps", bufs=1, space="PSUM") as sp:
                ssq_x = sp.tile([P, NBLK], F32)
                ssq_y = sp.tile([P, NBLK], F32)
                for j in range(NBLK):
                    nc.tensor.matmul(ssq_x[:, j:j + 1], xsq[:, j * P:(j + 1) * P],
                                     ones_col_bf[:], start=True, stop=True)
                for j in range(NBLK):
                    nc.tensor.matmul(ssq_y[:, j:j + 1], ysq[:, j * P:(j + 1) * P],
                                     ones_col_bf[:], start=True, stop=True)
                u_col = _norm_chain(nc, persist, ssq_x, "x")    # [128, 32] f32
                v_col = _norm_chain(nc, persist, ssq_y, "y")    # [128, 32] f32
            hu_col = persist.tile([P, NBLK], F32)               # -h * u
            nc.vector.tensor_scalar_mul(hu_col[:], u_col[:], -H_BW)

        # ---------- vrow broadcast + fold v into yc: yv = yc * vrow ----------
        yv = persist.tile([P, N], BF16)
        with tc.tile_pool(name="vrow_ps_pool", bufs=1, space="PSUM") as vp, \
             tc.tile_pool(name="diag_pool", bufs=1) as dp:
            vrow_ps = vp.tile([P, N], F32)
            diag_all = dp.tile([P, N], BF16)
            for j in range(NBLK):
                nc.vector.tensor_scalar_mul(diag_all[:, j * P:(j + 1) * P],
                                            ident_bf[:], v_col[:, j:j + 1])
            for j in range(NBLK):
                nc.tensor.matmul(vrow_ps[:, j * P:(j + 1) * P], ones_sq[:],
                                 diag_all[:, j * P:(j + 1) * P],
                                 start=True, stop=True)
            vrow_bf = dp.tile([P, N], BF16)
            nc.scalar.activation(vrow_bf[:], vrow_ps[:], AF.Copy)
            nc.vector.tensor_mul(yv[:], yc[:], vrow_bf[:])

        # ---------- main loop over 32 row blocks (pipelined by 1) ----------
        colmax = persist.tile([P, N], BF16)
        nc.vector.memset(colmax[:], 0.0)

        with tc.tile_pool(name="g_ps_pool", bufs=2, space="PSUM") as gp, \
             tc.tile_pool(name="sv_pool", bufs=2) as svp, \
             tc.tile_pool(name="w_pool", bufs=3) as wp, \
             tc.tile_pool(name="wn_pool", bufs=3) as wnp, \
             tc.tile_pool(name="fin", bufs=4) as fin:
            w_tiles = [None] * NBLK
            r_tiles = [None] * NBLK
            wn_tiles = [None] * NBLK

            def scale_w(j):
                # wn = w * (1/r): reciprocal on DVE, big scale on Pool
                rinv = fin.tile([P, 1], F32, name="rinv")
                nc.vector.reciprocal(rinv[:], r_tiles[j][:])
                wn = wnp.tile([P, N], BF16, name="wn")
                nc.gpsimd.tensor_scalar_mul(wn[:], w_tiles[j][:], rinv[:, 0:1])
                wn_tiles[j] = wn

            def max_w(j):
                # colmax = max(colmax, wn)  (DVE, bf16 2x)
                nc.vector.tensor_tensor(colmax[:], wn_tiles[j][:], colmax[:],
                                        OP.max)

            for nb in range(NBLK):
                lhsT = xc[:, nb * P:(nb + 1) * P]
                sv = svp.tile([P, N], BF16, name="sv")
                macc = small.tile([P, 1], F32, name="macc")
                gv = small.tile([P, 1], F32, name="gv")
                for ch in range(2):
                    g_ps = gp.tile([P, MC], F32, name="g_ps")
                    for j in range(MC // MM_N):
                        m0 = ch * MC + j * MM_N
                        nc.tensor.matmul(g_ps[:, j * MM_N:(j + 1) * MM_N],
                                         lhsT, yv[:, m0:m0 + MM_N],
                                         start=True, stop=True)
                    # fused PSUM->SBUF copy + row-max (frees PSUM banks now)
                    nc.vector._custom_dve(
                        TENSOR_MASK_REDUCE,
                        out=sv[:, ch * MC:(ch + 1) * MC],
                        in0=g_ps[:], in1=c_mend[:, 0:1],
                        s0=0.0,
                        s1=(NEG_INF if ch == 0 else macc[:, 0:1]),
                        imm2=1.0,
                        accum_out=(macc[:, 0:1] if ch == 0 else gv[:, 0:1]))

                # scalar chain: keep the small ops AHEAD of the big wn op in
                # the Pool queue so exp isn't gated behind it
                t = small.tile([P, 1], F32, name="t")
                nc.gpsimd.tensor_scalar(t[:], gv[:], hu_col[:, nb:nb + 1],
                                        c_hbias[:, 0:1], OP.mult, OP.add)
                sc = small.tile([P, 1], F32, name="sc")
                nc.vector.reciprocal(sc[:], t[:])
                scale_eff = small.tile([P, 1], F32, name="scale_eff")
                nc.gpsimd.tensor_scalar_mul(scale_eff[:], sc[:],
                                            u_col[:, nb:nb + 1])
                bias_v = small.tile([P, 1], F32, name="bias_v")
                nc.gpsimd.tensor_scalar(bias_v[:], sc[:], -1.0,
                                        c_invh[:, 0:1], OP.mult, OP.add)

                # w = exp(scale_eff*sv + bias_v); r = rowsum(w) fused
                w = wp.tile([P, N], BF16, name="w")
                r = fin.tile([P, 1], F32, name="r")
                nc.scalar.activation(w[:], sv[:], AF.Exp,
                                     bias=bias_v[:, 0:1],
                                     scale=scale_eff[:, 0:1],
                                     accum_out=r[:, 0:1])
                w_tiles[nb] = w
                r_tiles[nb] = r

                if nb > 0:
                    scale_w(nb - 1)
                if nb > 1:
                    max_w(nb - 2)

            scale_w(NBLK - 1)
            max_w(NBLK - 2)
            max_w(NBLK - 1)

        # ---------- tail: partition-max via PE transpose (bf16), mean+log ----
        cm_col = persist.tile([P, NBLK], F32)
        with tc.tile_pool(name="tail_ps", bufs=1, space="PSUM") as tp:
            cmT = tp.tile([P, N], BF16)     # 4 banks
            for j in range(NBLK):
                nc.tensor.transpose(cmT[:, j * P:(j + 1) * P],
                                    colmax[:, j * P:(j + 1) * P],
                                    ident_bf[:])
            nc.vector.reduce_max(cm_col[:],
                                 cmT[:].rearrange("p (j q) -> p j q", q=P),
                                 axis=mybir.AxisListType.X)
        cm_sum = persist.tile([P, 1], F32)
        nc.vector.reduce_sum(cm_sum[:], cm_col[:], axis=mybir.AxisListType.X)
        with tc.tile_pool(name="tot_ps", bufs=1, space="PSUM") as tp2:
            total = tp2.tile([1, 1], F32)
            nc.tensor.matmul(total[:], cm_sum[:], ones_col_f[:],
                             start=True, stop=True)
            lnv = persist.tile([1, 1], F32)
            nc.scalar.activation(lnv[:], total[:], AF.Ln,
                                 bias=c_eps[0:1, 0:1], scale=1.0 / N)
            loss_sb = persist.tile([1, 1], F32)
            nc.vector.tensor_scalar_mul(loss_sb[:], lnv[:], -1.0)
            nc.sync.dma_start(loss_d, loss_sb[:])


_NC_CACHE = None


def _get_nc():
    global _NC_CACHE
    if _NC_CACHE is None:
        nc = bacc.Bacc("TRN2", target_bir_lowering=False, debug=False)
        with tile.TileContext(nc) as tc:
            _kernel_body(tc)
        nc.compile()
        _NC_CACHE = nc
    return _NC_CACHE


def kernel(inputs, targets):
    x = np.ascontiguousarray(np.asarray(inputs, dtype=np.float32))
    y = np.ascontiguousarray(np.asarray(targets, dtype=np.float32))
    assert x.shape == (B, C, H, W) and y.shape == (B, C, H, W)
    mu = y.mean(axis=(0, 2, 3)).astype(np.float32).reshape(C, 1)
    in_maps = [
        {
            "x": x[b].reshape(C, N),
            "y": y[b].reshape(C, N),
            "mu": mu,
            "ident": np.eye(P, dtype=np.float32),
        }
        for b in range(B)
    ]
    nc = _get_nc()
    res = run_bass_kernel_spmd(nc, in_maps, list(range(N_CORES)))
    losses = [float(res.results[b]["loss"][0, 0]) for b in range(B)]
    return np.float32(np.mean(losses))
